# revision 1
# baseline (speedup 1.0000x reference)
"""BERT embedding lookup on 8 TRN2 NeuronCores.

Strategy: data-parallel over batch — core c handles batch rows [4c, 4c+4)
(2048 tokens, 16 tiles of 128); no collectives. Per tile:
  1. SWDGE indirect DMA gathers the 128 word-embedding rows
     (one row index per partition; 16 gathers alternate across two
     SWDGE queues)
  2. DVE computes base = posA[s-block] + tt * diff (scalar_tensor_tensor
     with the per-partition token-type as the scalar), then adds the
     gathered rows
  3. HWDGE store of the finished [128, 768] tile
posA = position_embedding + type_row0 and diff = type_row1 - type_row0
are host-precomputed (cheap O(S*H) work). Constants travel in one blob;
the index tile loads first so gathers start early, and the blob half
needed by the first tiles (ttf, diff, pos0) loads before the rest.
All f32. Measured ~53-54 us on hardware; the steady phase runs at the
per-core DMA-fabric limit (~340 GB/s for this random-read mix).
"""

import numpy as np

P = 128
H = 768
VOCAB = 30522
SEQ = 512
BATCH = 32
N_CORES = 8
TOK_PER_CORE = BATCH * SEQ // N_CORES  # 2048
T_TILES = TOK_PER_CORE // P  # 16
S_BLOCKS = SEQ // P  # 4
# blob layout: ttf(16) | diff(768) | posA(4*768); ttf+diff+pos0 load first
FBLOB_W = T_TILES + H + S_BLOCKS * H
FB_SPLIT = T_TILES + H + H  # first DMA covers ttf, diff, pos0

_CACHE = {}


def _build(wt_bufs=10, res_bufs=8):
    from concourse import bacc, mybir
    import concourse.bass as bass
    import concourse.tile as tile

    nc = bacc.Bacc(
        "TRN2",
        target_bir_lowering=False,
        debug=False,
        num_devices=N_CORES,
        dynamic_dma_scratch_size=65536,
        num_swdge_queues=2,
    )
    f32 = mybir.dt.float32
    i32 = mybir.dt.int32

    wemb = nc.dram_tensor("wemb", [VOCAB, H], f32, kind="ExternalInput").ap()
    fblob = nc.dram_tensor("fblob", [P, FBLOB_W], f32, kind="ExternalInput").ap()
    ids = nc.dram_tensor("ids", [P, T_TILES], i32, kind="ExternalInput").ap()
    out = nc.dram_tensor("out", [TOK_PER_CORE, H], f32, kind="ExternalOutput").ap()

    with tile.TileContext(nc) as tc:
        with (
            tc.tile_pool(name="consts", bufs=1) as consts,
            tc.tile_pool(name="wtp", bufs=wt_bufs) as wpool,
            tc.tile_pool(name="res", bufs=res_bufs) as rpool,
        ):
            ids_sb = consts.tile([P, T_TILES], i32)
            nc.sync.dma_start(out=ids_sb[:], in_=ids[:])
            fb = consts.tile([P, FBLOB_W], f32)
            nc.scalar.dma_start(out=fb[:, :FB_SPLIT], in_=fblob[:, :FB_SPLIT])
            nc.scalar.dma_start(out=fb[:, FB_SPLIT:], in_=fblob[:, FB_SPLIT:])
            ttf_sb = fb[:, :T_TILES]
            diff_sb = fb[:, T_TILES : T_TILES + H]
            pos0 = T_TILES + H
            pos_sb = [fb[:, pos0 + sb * H : pos0 + (sb + 1) * H] for sb in range(S_BLOCKS)]

            for t in range(T_TILES):
                wt = wpool.tile([P, H], f32)
                gi = nc.gpsimd.indirect_dma_start(
                    out=wt[:],
                    out_offset=None,
                    in_=wemb[:],
                    in_offset=bass.IndirectOffsetOnAxis(
                        ap=ids_sb[:, t : t + 1], axis=0
                    ),
                )
                if t % 2 == 1:
                    gi.ins.queue = "qPoolDynamic1"
                res = rpool.tile([P, H], f32)
                # res = diff * tt + posA[s-block]
                nc.vector.scalar_tensor_tensor(
                    out=res[:],
                    in0=diff_sb,
                    scalar=ttf_sb[:, t : t + 1],
                    in1=pos_sb[t % S_BLOCKS],
                    op0=mybir.AluOpType.mult,
                    op1=mybir.AluOpType.add,
                )
                nc.vector.tensor_add(out=res[:], in0=res[:], in1=wt[:])
                nc.sync.dma_start(out=out[t * P : (t + 1) * P, :], in_=res[:])

    nc.compile()
    return nc


def _get_nc():
    if "nc" not in _CACHE:
        _CACHE["nc"] = _build()
    return _CACHE["nc"]


def _prep_inputs(
    input_ids, token_type_ids, word_embedding, position_embedding, token_type_embedding
):
    ids = np.ascontiguousarray(
        np.asarray(input_ids, dtype=np.int32)
        .reshape(N_CORES, T_TILES, P)
        .transpose(0, 2, 1)
    )
    ttf = (
        np.asarray(token_type_ids, dtype=np.float32)
        .reshape(N_CORES, T_TILES, P)
        .transpose(0, 2, 1)
    )  # [N, 128, 16]
    wemb = np.ascontiguousarray(np.asarray(word_embedding, dtype=np.float32))
    pos = np.asarray(position_embedding, dtype=np.float32)
    typ = np.asarray(token_type_embedding, dtype=np.float32)
    posA = (pos + typ[0][None, :]).reshape(S_BLOCKS, P, H).transpose(1, 0, 2)
    posA = posA.reshape(P, S_BLOCKS * H)  # [128, 4*768], col-block sb = posA[sb*128+p]
    diffr = np.broadcast_to(typ[1] - typ[0], (P, H))
    fblob = np.empty((N_CORES, P, FBLOB_W), dtype=np.float32)
    fblob[:, :, :T_TILES] = ttf
    fblob[:, :, T_TILES : T_TILES + H] = diffr[None]
    fblob[:, :, T_TILES + H :] = posA[None]
    return [
        {"wemb": wemb, "fblob": np.ascontiguousarray(fblob[c]), "ids": ids[c]}
        for c in range(N_CORES)
    ]


def kernel(
    input_ids, token_type_ids, word_embedding, position_embedding, token_type_embedding
):
    from concourse.bass_utils import run_bass_kernel_spmd

    nc = _get_nc()
    in_maps = _prep_inputs(
        input_ids,
        token_type_ids,
        word_embedding,
        position_embedding,
        token_type_embedding,
    )
    r = run_bass_kernel_spmd(nc, in_maps, core_ids=list(range(N_CORES)))
    out = np.stack([r.results[c]["out"] for c in range(N_CORES)], axis=0)
    return out.reshape(BATCH, SEQ, H)

